# revision 20
# baseline (speedup 1.0000x reference)
"""CWRNN-LM Trainium2 kernel: 8-core SPMD, replicated clockwork-RNN scan +
vocab-sharded output projection with on-device sharded log-softmax stats.

Self-contained: hardcodes all shapes from the problem spec.
  B=16, T1=257 (T=256 steps), EMB=512, NH=1024 (4 clock blocks x 256),
  VOCAB=32000, periods (1,2,4,8).

v3: staged-lagged clockwork scan, everything SBUF-resident.
  - Each clock block b runs its own chain of n_b = 256>>b tanh steps into a
    linear SBUF history S_b [128, 2, n_b, 16] (partition=unit%128,
    c=chunk-in-block, chain index, batch).
  - Pre-activations accumulate in a per-block PSUM window: U-matmuls
    (embT @ Wi + bias via K=1 ones-matmul) fill the window, cross-block
    contributions and the self-recurrence accumulate per step, one tanh
    per chain step.
  - Block b>0 lags block b-1 so all chains run concurrently inside block
    0's 256-slot wall; cross-block matmuls read lagged history directly.
  - h(t) row tiles for the projection are assembled by DVE broadcast
    copies (0-stride repeats) into [128, 8, 8, 16] tiles; projection
    k-sweeps interleave between scan steps; drains emit raw logits (f32),
    -rowmax, sum(exp(l-max)) per 1000-vocab group.
Host: embedding gather + input folding up front; tiny log-softmax combine
+ target gather + masked loss at the end.
"""
import sys

sys.path.insert(0, "/opt/trn_rl_repo")

import numpy as np
import ml_dtypes

import concourse.bass as bass
import concourse.mybir as mybir
import concourse.tile as tile
from concourse import bacc, bass_utils

F32 = mybir.dt.float32
BF16 = mybir.dt.bfloat16
AF = mybir.ActivationFunctionType
AX = mybir.AxisListType

B = 16
T = 256
EMB = 512
NH = 1024
VOCAB = 32000
NCORES = 8
VS = VOCAB // NCORES          # 4000 vocab per core
ROWS = B * T                  # 4096 (row r = t*16 + b)
MT = ROWS // 128              # 32 row tiles
NJ = NH // 128                # 8 unit chunks
NE = EMB // 128               # 4 emb chunks
GW = 1000                     # vocab group width (4 groups/core)
NG = VS // GW                 # 4
PROJ_CHUNKS = ((0, 512), (512, 488))

CH = [T >> b for b in range(4)]        # chain lengths 256,128,64,32
WLEN = [16, 8, 8, 4]                   # psum window length (chain steps)

# whb packing: out-chunk jo receives in-chunks ji <= 2*(jo//2)+1
WH_PAIRS = [(jo, ji) for jo in range(NJ) for ji in range(2 * (jo // 2) + 2)]
WH_IDX = {p: i for i, p in enumerate(WH_PAIRS)}


def _build():
    nc = bacc.Bacc("TRN2", target_bir_lowering=False, debug=False)

    embt_d = nc.dram_tensor("embt", [NE, 128, ROWS], BF16, kind="ExternalInput")
    wi_d = nc.dram_tensor("wi", [NE, 128, NH], BF16, kind="ExternalInput")
    bsumb_d = nc.dram_tensor("bsumb", [1, NJ, 128], BF16, kind="ExternalInput")
    ones_d = nc.dram_tensor("ones", [1, 256], BF16, kind="ExternalInput")
    whb_d = nc.dram_tensor("whb", [len(WH_PAIRS), 128, 128], BF16, kind="ExternalInput")
    hinit_d = nc.dram_tensor("hinit", [128, NJ, B], BF16, kind="ExternalInput")
    wo_d = nc.dram_tensor("wo", [NJ, 128, VS], BF16, kind="ExternalInput")

    logits_d = nc.dram_tensor("logits", [ROWS, VS], F32, kind="ExternalOutput")
    stats_d = nc.dram_tensor("stats", [MT, 128, 2 * NG], F32, kind="ExternalOutput")

    with tile.TileContext(nc) as tc:
        with (
            tc.tile_pool(name="weights", bufs=1) as wpool,
            tc.tile_pool(name="hist", bufs=1) as histpool,
            tc.tile_pool(name="asm", bufs=4) as asmpool,
            tc.tile_pool(name="lg", bufs=3) as lgpool,
            tc.tile_pool(name="stats", bufs=3) as stpool,
            tc.tile_pool(name="trash", bufs=1) as trpool,
            tc.tile_pool(name="scanps", bufs=1, space="PSUM") as scps,
            tc.tile_pool(name="projps", bufs=2, space="PSUM") as pps,
        ):
            # ---- inputs ordered so scan-gating data arrives first ----
            wi = wpool.tile([128, NE, NH], BF16)
            for e in range(NE):
                nc.sync.dma_start(wi[:, e, 0:256], wi_d[e][:, 0:256])
            for e in range(NE):
                nc.sync.dma_start(wi[:, e, 256:NH], wi_d[e][:, 256:NH])
            bsumb = wpool.tile([1, NJ, 128], BF16)
            nc.sync.dma_start(bsumb[:], bsumb_d[:])
            ones = wpool.tile([1, 256], BF16)
            nc.sync.dma_start(ones[:], ones_d[:])
            hinit = wpool.tile([128, NJ, B], BF16)
            nc.sync.dma_start(hinit[:], hinit_d[:])
            embt = wpool.tile([128, NE, ROWS], BF16)
            for e in range(NE):                      # first 2 windows' columns
                nc.sync.dma_start(embt[:, e, 0:512], embt_d[e][:, 0:512])
            whb = wpool.tile([128, len(WH_PAIRS), 128], BF16)
            for i in range(len(WH_PAIRS)):
                nc.sync.dma_start(whb[:, i, :], whb_d[i])
            for e in range(NE):                      # rest of the sequence
                nc.sync.dma_start(embt[:, e, 512:ROWS], embt_d[e][:, 512:ROWS])
            trash = trpool.tile([128, GW], BF16)

            # block chain histories (linear, write-once per index)
            S = [
                histpool.tile([128, 2, CH[b], B], BF16, tag=f"S{b}", name=f"S{b}")
                for b in range(4)
            ]
            # embt grouped by (tau, rep, batch) per block for U matmuls
            embt_g = [
                embt[:, :, :].rearrange("p e (tau r b) -> p e tau r b", r=1 << b, b=B)
                for b in range(4)
            ]
            # odd-index view of S0 history for block1's windowed cross matmuls
            S0_odd = S[0].rearrange("p c (x two) b -> p c x two b", two=2)

            pwin = [None] * 4

            def src(a, c, idx):
                """h_a chunk c at chain index idx (hinit when idx < 0)."""
                if idx < 0:
                    return hinit[:, 2 * a + c, :]
                return S[a][:, c, idx, :]

            def emit_window(b, w):
                """Allocate block-b psum window w and fill with U + bias."""
                L = WLEN[b]
                ps = scps.tile([128, 2, L, B], F32, tag=f"P{b}", name=f"P{b}")
                pwin[b] = ps
                t0 = w * L          # chain index of first step in window
                for c in range(2):
                    j = 2 * b + c
                    for e in range(NE):
                        nc.tensor.matmul(
                            ps[:, c, :, :],
                            wi[:, e, j * 128:(j + 1) * 128],
                            embt_g[b][:, e, t0:t0 + L, 0, :],
                            start=(c == 0 and e == 0),
                            stop=False,
                            skip_group_check=True,
                        )
                    nc.tensor.matmul(
                        ps[:, c, :, :],
                        bsumb[:, j, :],
                        ones[:, 0:L * B],
                        start=False, stop=False, skip_group_check=True,
                    )
                return ps

            def emit_cross_window_b1(w):
                """Block1 window-w cross contributions from block0 history."""
                ps = pwin[1]
                for c_out in range(2):
                    for c_in in range(2):
                        lhsT = whb[:, WH_IDX[(2 + c_out, c_in)], :]
                        if w == 0:
                            nc.tensor.matmul(
                                ps[:, c_out, 0, :], lhsT, hinit[:, c_in, :],
                                start=False, stop=False, skip_group_check=True,
                            )
                            nc.tensor.matmul(
                                ps[:, c_out, 1:8, :], lhsT,
                                S0_odd[:, c_in, 0:7, 1, :],
                                start=False, stop=False, skip_group_check=True,
                            )
                        else:
                            nc.tensor.matmul(
                                ps[:, c_out, :, :], lhsT,
                                S0_odd[:, c_in, 8 * w - 1:8 * w + 7, 1, :],
                                start=False, stop=False, skip_group_check=True,
                            )

            def emit_step(b, tau):
                """One chain step of block b (cross for b>=2 + self + tanh)."""
                L = WLEN[b]
                i = tau % L
                ps = pwin[b]
                if b >= 2:
                    for a in range(b):
                        idx = (tau << (b - a)) - 1
                        for c_out in range(2):
                            for c_in in range(2):
                                nc.tensor.matmul(
                                    ps[:, c_out, i, :],
                                    whb[:, WH_IDX[(2 * b + c_out, 2 * a + c_in)], :],
                                    src(a, c_in, idx),
                                    start=False, stop=False, skip_group_check=True,
                                )
                for c_out in range(2):
                    for c_in in range(2):
                        nc.tensor.matmul(
                            ps[:, c_out, i, :],
                            whb[:, WH_IDX[(2 * b + c_out, 2 * b + c_in)], :],
                            src(b, c_in, tau - 1),
                            start=False, stop=(c_in == 1), skip_group_check=True,
                        )
                nc.scalar.activation(S[b][:, :, tau, :], ps[:, :, i, :], AF.Tanh)

            asm_tiles = {}

            def emit_asm(m):
                """Assemble h rows t in [8m, 8m+8) into [128, NJ, 8, B]."""
                a = asmpool.tile([128, NJ, 8, B], BF16, tag="asm", name="asm")
                for j in range(NJ):
                    b, c = j // 2, j % 2
                    rep = 1 << b
                    n = 8 >> b
                    dst = a[:, j, :, :].rearrange("p (n r) x -> p n r x", r=rep)
                    s = S[b][:, c, m * n:(m + 1) * n, :]
                    nc.vector.tensor_copy(
                        dst, s[:, :, None, :].broadcast_to([128, n, rep, B])
                    )
                asm_tiles[m] = a

            class ProjEmitter:
                def __init__(self):
                    self.m = 0
                    self.g = 0
                    self.k = 0
                    self.st = None
                    self.ps = None

                def next_m(self):
                    return self.m if self.m < MT else None

                def emit_sweep(self):
                    m, g, k = self.m, self.g, self.k
                    if g == 0 and k == 0:
                        emit_asm(m)
                        self.st = stpool.tile([128, 2 * NG], F32, tag="st", name="st")
                    if k == 0:
                        self.ps = pps.tile([128, 1024], F32, tag="pps", name="pps")
                    hts = asm_tiles[m]
                    for off, wd in PROJ_CHUNKS:
                        nc.tensor.matmul(
                            self.ps[:, off:off + wd],
                            hts[:, k, :, :],
                            wo[:, k, g * GW + off:g * GW + off + wd],
                            start=(k == 0),
                            stop=(k == NJ - 1),
                        )
                    self.k += 1
                    if self.k == NJ:
                        self.k = 0
                        lg = lgpool.tile([128, GW], F32, tag="lg", name="lg")
                        nc.vector.tensor_copy(lg[:], self.ps[:, 0:GW])
                        nc.vector.reduce_max(
                            self.st[:, 2 * g:2 * g + 1], lg[:], axis=AX.X, negate=True
                        )
                        nc.scalar.activation(
                            trash[:], lg[:], AF.Exp,
                            bias=self.st[:, 2 * g:2 * g + 1],
                            accum_out=self.st[:, 2 * g + 1:2 * g + 2],
                        )
                        nc.sync.dma_start(
                            logits_d[m * 128:(m + 1) * 128, g * GW:(g + 1) * GW], lg[:]
                        )
                        self.g += 1
                        if self.g == NG:
                            self.g = 0
                            nc.sync.dma_start(stats_d[m], self.st[:])
                            asm_tiles.pop(m, None)
                            self.m += 1

            pe = ProjEmitter()

            # Wo loads gate only the projection; emit them after the small
            # inputs so embt/wi win the DMA queues.
            wo = wpool.tile([128, NJ, VS], BF16)
            for j in range(NJ):
                nc.sync.dma_start(wo[:, j, :], wo_d[j])

            done_m = -1
            for s in range(0, 274):
                # block 0: one step per slot
                if s < T:
                    if s % WLEN[0] == 0:
                        emit_window(0, s // WLEN[0])
                    emit_step(0, s)
                if s >= 16:
                    d = s - 16
                    # block 1: every 2 slots, windowed cross from S0
                    if d % 2 == 0 and d // 2 < CH[1]:
                        tau = d // 2
                        if tau % 8 == 0:
                            emit_window(1, tau // 8)
                            emit_cross_window_b1(tau // 8)
                        emit_step(1, tau)
                    # block 2: every 4 slots
                    if d % 4 == 0 and d // 4 < CH[2]:
                        tau = d // 4
                        if tau % 8 == 0:
                            emit_window(2, tau // 8)
                        emit_step(2, tau)
                    # block 3: every 8 slots
                    if d % 8 == 0 and d // 8 < CH[3]:
                        tau = d // 8
                        if tau % 4 == 0:
                            emit_window(3, tau // 4)
                        emit_step(3, tau)
                if s >= 24 and (s - 24) % 8 == 0 and (s - 24) // 8 < MT:
                    done_m = (s - 24) // 8
                for _ in range(2):
                    if pe.next_m() is None or pe.next_m() > done_m:
                        break
                    pe.emit_sweep()
            while pe.next_m() is not None:
                pe.emit_sweep()

    nc.compile()
    return nc


_NC_CACHE = {}
LAST_RESULT = {}


def _get_nc():
    if "nc" not in _NC_CACHE:
        _NC_CACHE["nc"] = _build()
    return _NC_CACHE["nc"]


def _maybe_trace_kwargs():
    """When KERNEL_TRACE=1, enable NTFF profiling (dev/test only)."""
    import os
    if os.environ.get("KERNEL_TRACE") != "1":
        return {}
    try:
        import types
        if "antenv.axon_hooks" not in sys.modules:
            mod = types.ModuleType("antenv.axon_hooks")
            state = {"hook": None}
            mod.set_axon_ntff_profile_hook = lambda h: state.__setitem__("hook", h)
            mod.get_axon_ntff_profile_hook = lambda: state["hook"]
            sys.modules["antenv.axon_hooks"] = mod
            from trn_agent_boot.trn_boot import _ntff_profile_via_ctypes
            mod.set_axon_ntff_profile_hook(
                _ntff_profile_via_ctypes("/opt/axon/libaxon_pjrt.so")
            )
        return {"trace": True}
    except Exception:
        return {}


def kernel(x, x_sl, Wi, Wh, Wo, bi, bh, initial_state, embedding):
    x = np.asarray(x)
    x_sl = np.asarray(x_sl)
    Wi = np.asarray(Wi, np.float32)
    Wh = np.asarray(Wh, np.float32)
    Wo = np.asarray(Wo, np.float32)
    bi = np.asarray(bi, np.float32)
    bh = np.asarray(bh, np.float32)
    initial_state = np.asarray(initial_state, np.float32)
    embedding = np.asarray(embedding, np.float32)

    xin = x[:, :-1].astype(np.int64)
    y = x[:, 1:].astype(np.int64)
    sl = (x_sl - 1).astype(np.int64)

    bf16 = ml_dtypes.bfloat16
    emb = embedding[xin]                                   # [B,T,E] f32
    embt = np.ascontiguousarray(
        emb.transpose(2, 1, 0).reshape(EMB, ROWS)          # col = t*16+b
    ).reshape(NE, 128, ROWS).astype(bf16)
    wi_h = Wi.reshape(NE, 128, NH).astype(bf16)
    bsumb_h = (bi + bh).reshape(1, NJ, 128).astype(bf16)
    whb_h = np.stack(
        [Wh[ji * 128:(ji + 1) * 128, jo * 128:(jo + 1) * 128] for jo, ji in WH_PAIRS]
    ).astype(bf16)
    hinit_h = np.ascontiguousarray(
        np.broadcast_to(initial_state.reshape(NJ, 128).T[:, :, None], (128, NJ, B))
    ).astype(bf16)
    wo_r = Wo.reshape(NJ, 128, VOCAB).astype(bf16)

    base = {
        "embt": embt, "wi": wi_h, "bsumb": bsumb_h, "whb": whb_h, "hinit": hinit_h,
        "ones": np.ones((1, 256), dtype=bf16),
    }
    in_maps = [
        {**base, "wo": np.ascontiguousarray(wo_r[:, :, c * VS:(c + 1) * VS])}
        for c in range(NCORES)
    ]

    nc = _get_nc()
    res = bass_utils.run_bass_kernel_spmd(
        nc, in_maps, core_ids=list(range(NCORES)), **_maybe_trace_kwargs()
    )
    LAST_RESULT["res"] = res

    logits_rows = np.concatenate(
        [res.results[c]["logits"] for c in range(NCORES)], axis=1
    )                                                      # [ROWS, VOCAB]
    stats = np.stack([res.results[c]["stats"] for c in range(NCORES)])
    # stats[c, m, p, 2g] = -max, [.., 2g+1] = sumexp ; row = m*128+p
    negmax = stats[:, :, :, 0::2].reshape(NCORES, ROWS, NG)
    sume = stats[:, :, :, 1::2].reshape(NCORES, ROWS, NG)
    M_cg = -negmax.astype(np.float64)
    S_cg = sume.astype(np.float64)
    M = M_cg.max(axis=(0, 2))                              # [ROWS]
    Ssum = (np.exp(M_cg - M[None, :, None]) * S_cg).sum(axis=(0, 2))
    lse = M + np.log(Ssum)                                 # [ROWS]

    y_row = y.T.reshape(ROWS)                              # row = t*16+b
    tgt = logits_rows[np.arange(ROWS), y_row].astype(np.float64)
    ll = tgt - lse
    t_idx = np.arange(ROWS) // B
    b_idx = np.arange(ROWS) % B
    mask = t_idx < sl[b_idx]
    loss = np.float32(-(ll * mask).sum() / sl.sum())

    logits = np.ascontiguousarray(
        logits_rows.reshape(T, B, VOCAB).transpose(1, 0, 2)
    )
    return loss, logits


# revision 21
# speedup vs baseline: 1.0423x; 1.0423x over previous
"""CWRNN-LM Trainium2 kernel: 8-core SPMD, replicated clockwork-RNN scan +
vocab-sharded output projection with on-device sharded log-softmax stats.

Self-contained: hardcodes all shapes from the problem spec.
  B=16, T1=257 (T=256 steps), EMB=512, NH=1024 (4 clock blocks x 256),
  VOCAB=32000, periods (1,2,4,8).

v3: staged-lagged clockwork scan, everything SBUF-resident.
  - Each clock block b runs its own chain of n_b = 256>>b tanh steps into a
    linear SBUF history S_b [128, 2, n_b, 16] (partition=unit%128,
    c=chunk-in-block, chain index, batch).
  - Pre-activations accumulate in a per-block PSUM window: U-matmuls
    (embT @ Wi + bias via K=1 ones-matmul) fill the window, cross-block
    contributions and the self-recurrence accumulate per step, one tanh
    per chain step.
  - Block b>0 lags block b-1 so all chains run concurrently inside block
    0's 256-slot wall; cross-block matmuls read lagged history directly.
  - h(t) row tiles for the projection are assembled by DVE broadcast
    copies (0-stride repeats) into [128, 8, 8, 16] tiles; projection
    k-sweeps interleave between scan steps; drains emit raw logits (f32),
    -rowmax, sum(exp(l-max)) per 1000-vocab group.
Host: embedding gather + input folding up front; tiny log-softmax combine
+ target gather + masked loss at the end.
"""
import sys

sys.path.insert(0, "/opt/trn_rl_repo")

import numpy as np
import ml_dtypes

import concourse.bass as bass
import concourse.mybir as mybir
import concourse.tile as tile
from concourse import bacc, bass_utils

F32 = mybir.dt.float32
BF16 = mybir.dt.bfloat16
AF = mybir.ActivationFunctionType
AX = mybir.AxisListType

B = 16
T = 256
EMB = 512
NH = 1024
VOCAB = 32000
NCORES = 8
VS = VOCAB // NCORES          # 4000 vocab per core
ROWS = B * T                  # 4096 (row r = t*16 + b)
MT = ROWS // 128              # 32 row tiles
NJ = NH // 128                # 8 unit chunks
NE = EMB // 128               # 4 emb chunks
GW = 1000                     # vocab group width (4 groups/core)
NG = VS // GW                 # 4
PROJ_CHUNKS = ((0, 512), (512, 488))

CH = [T >> b for b in range(4)]        # chain lengths 256,128,64,32
WLEN = [16, 8, 8, 4]                   # psum window length (chain steps)

# whb packing: out-chunk jo receives in-chunks ji <= 2*(jo//2)+1
WH_PAIRS = [(jo, ji) for jo in range(NJ) for ji in range(2 * (jo // 2) + 2)]
WH_IDX = {p: i for i, p in enumerate(WH_PAIRS)}


def _build(use_bias=True):
    nc = bacc.Bacc("TRN2", target_bir_lowering=False, debug=False)

    embt_d = nc.dram_tensor("embt", [NE, 128, ROWS], BF16, kind="ExternalInput")
    wi_d = nc.dram_tensor("wi", [NE, 128, NH], BF16, kind="ExternalInput")
    bsumb_d = nc.dram_tensor("bsumb", [1, NJ, 128], BF16, kind="ExternalInput")
    ones_d = nc.dram_tensor("ones", [1, 256], BF16, kind="ExternalInput")
    whb_d = nc.dram_tensor("whb", [len(WH_PAIRS), 128, 128], BF16, kind="ExternalInput")
    hinit_d = nc.dram_tensor("hinit", [128, NJ, B], BF16, kind="ExternalInput")
    wo_d = nc.dram_tensor("wo", [NJ, 128, VS], BF16, kind="ExternalInput")

    logits_d = nc.dram_tensor("logits", [ROWS, VS], F32, kind="ExternalOutput")
    stats_d = nc.dram_tensor("stats", [MT, 128, 2 * NG], F32, kind="ExternalOutput")

    with tile.TileContext(nc) as tc:
        with (
            tc.tile_pool(name="weights", bufs=1) as wpool,
            tc.tile_pool(name="hist", bufs=1) as histpool,
            tc.tile_pool(name="asm", bufs=4) as asmpool,
            tc.tile_pool(name="lg", bufs=3) as lgpool,
            tc.tile_pool(name="stats", bufs=3) as stpool,
            tc.tile_pool(name="trash", bufs=1) as trpool,
            tc.tile_pool(name="scanps", bufs=1, space="PSUM") as scps,
            tc.tile_pool(name="projps", bufs=2, space="PSUM") as pps,
        ):
            # ---- inputs ordered so scan-gating data arrives first ----
            wi = wpool.tile([128, NE, NH], BF16)
            for e in range(NE):
                nc.sync.dma_start(wi[:, e, 0:256], wi_d[e][:, 0:256])
            for e in range(NE):
                nc.sync.dma_start(wi[:, e, 256:NH], wi_d[e][:, 256:NH])
            bsumb = wpool.tile([1, NJ, 128], BF16)
            nc.sync.dma_start(bsumb[:], bsumb_d[:])
            ones = wpool.tile([1, 256], BF16)
            nc.sync.dma_start(ones[:], ones_d[:])
            hinit = wpool.tile([128, NJ, B], BF16)
            nc.sync.dma_start(hinit[:], hinit_d[:])
            embt = wpool.tile([128, NE, ROWS], BF16)
            for e in range(NE):                      # first 2 windows' columns
                nc.sync.dma_start(embt[:, e, 0:512], embt_d[e][:, 0:512])
            whb = wpool.tile([128, len(WH_PAIRS), 128], BF16)
            for i in range(len(WH_PAIRS)):
                nc.sync.dma_start(whb[:, i, :], whb_d[i])
            for e in range(NE):                      # rest of the sequence
                nc.sync.dma_start(embt[:, e, 512:ROWS], embt_d[e][:, 512:ROWS])
            trash = trpool.tile([128, GW], BF16)

            # block chain histories (linear, write-once per index)
            S = [
                histpool.tile([128, 2, CH[b], B], BF16, tag=f"S{b}", name=f"S{b}")
                for b in range(4)
            ]
            # embt grouped by (tau, rep, batch) per block for U matmuls
            embt_g = [
                embt[:, :, :].rearrange("p e (tau r b) -> p e tau r b", r=1 << b, b=B)
                for b in range(4)
            ]
            # odd-index view of S0 history for block1's windowed cross matmuls
            S0_odd = S[0].rearrange("p c (x two) b -> p c x two b", two=2)

            pwin = [None] * 4

            def src(a, c, idx):
                """h_a chunk c at chain index idx (hinit when idx < 0)."""
                if idx < 0:
                    return hinit[:, 2 * a + c, :]
                return S[a][:, c, idx, :]

            def emit_window(b, w):
                """Allocate block-b psum window w and fill with U + bias."""
                L = WLEN[b]
                ps = scps.tile([128, 2, L, B], F32, tag=f"P{b}", name=f"P{b}")
                pwin[b] = ps
                t0 = w * L          # chain index of first step in window
                for c in range(2):
                    j = 2 * b + c
                    for e in range(NE):
                        nc.tensor.matmul(
                            ps[:, c, :, :],
                            wi[:, e, j * 128:(j + 1) * 128],
                            embt_g[b][:, e, t0:t0 + L, 0, :],
                            start=(c == 0 and e == 0),
                            stop=False,
                            skip_group_check=True,
                        )
                    if use_bias:
                        nc.tensor.matmul(
                            ps[:, c, :, :],
                            bsumb[:, j, :],
                            ones[:, 0:L * B],
                            start=False, stop=False, skip_group_check=True,
                        )
                return ps

            def emit_cross_window_b1(w):
                """Block1 window-w cross contributions from block0 history."""
                ps = pwin[1]
                for c_out in range(2):
                    for c_in in range(2):
                        lhsT = whb[:, WH_IDX[(2 + c_out, c_in)], :]
                        if w == 0:
                            nc.tensor.matmul(
                                ps[:, c_out, 0, :], lhsT, hinit[:, c_in, :],
                                start=False, stop=False, skip_group_check=True,
                            )
                            nc.tensor.matmul(
                                ps[:, c_out, 1:8, :], lhsT,
                                S0_odd[:, c_in, 0:7, 1, :],
                                start=False, stop=False, skip_group_check=True,
                            )
                        else:
                            nc.tensor.matmul(
                                ps[:, c_out, :, :], lhsT,
                                S0_odd[:, c_in, 8 * w - 1:8 * w + 7, 1, :],
                                start=False, stop=False, skip_group_check=True,
                            )

            def emit_step(b, tau):
                """One chain step of block b (cross for b>=2 + self + tanh)."""
                L = WLEN[b]
                i = tau % L
                ps = pwin[b]
                if b >= 2:
                    for a in range(b):
                        idx = (tau << (b - a)) - 1
                        for c_out in range(2):
                            for c_in in range(2):
                                nc.tensor.matmul(
                                    ps[:, c_out, i, :],
                                    whb[:, WH_IDX[(2 * b + c_out, 2 * a + c_in)], :],
                                    src(a, c_in, idx),
                                    start=False, stop=False, skip_group_check=True,
                                )
                for c_out in range(2):
                    for c_in in range(2):
                        nc.tensor.matmul(
                            ps[:, c_out, i, :],
                            whb[:, WH_IDX[(2 * b + c_out, 2 * b + c_in)], :],
                            src(b, c_in, tau - 1),
                            start=False, stop=(c_in == 1), skip_group_check=True,
                        )
                nc.scalar.activation(S[b][:, :, tau, :], ps[:, :, i, :], AF.Tanh)

            asm_tiles = {}

            def emit_asm(m):
                """Assemble h rows t in [8m, 8m+8) into [128, NJ, 8, B]."""
                a = asmpool.tile([128, NJ, 8, B], BF16, tag="asm", name="asm")
                for j in range(NJ):
                    b, c = j // 2, j % 2
                    rep = 1 << b
                    n = 8 >> b
                    dst = a[:, j, :, :].rearrange("p (n r) x -> p n r x", r=rep)
                    s = S[b][:, c, m * n:(m + 1) * n, :]
                    nc.vector.tensor_copy(
                        dst, s[:, :, None, :].broadcast_to([128, n, rep, B])
                    )
                asm_tiles[m] = a

            class ProjEmitter:
                def __init__(self):
                    self.m = 0
                    self.g = 0
                    self.k = 0
                    self.st = None
                    self.ps = None

                def next_m(self):
                    return self.m if self.m < MT else None

                def emit_sweep(self):
                    m, g, k = self.m, self.g, self.k
                    if g == 0 and k == 0:
                        emit_asm(m)
                        self.st = stpool.tile([128, 2 * NG], F32, tag="st", name="st")
                    if k == 0:
                        self.ps = pps.tile([128, 1024], F32, tag="pps", name="pps")
                    hts = asm_tiles[m]
                    for off, wd in PROJ_CHUNKS:
                        nc.tensor.matmul(
                            self.ps[:, off:off + wd],
                            hts[:, k, :, :],
                            wo[:, k, g * GW + off:g * GW + off + wd],
                            start=(k == 0),
                            stop=(k == NJ - 1),
                        )
                    self.k += 1
                    if self.k == NJ:
                        self.k = 0
                        lg = lgpool.tile([128, GW], F32, tag="lg", name="lg")
                        nc.vector.tensor_copy(lg[:], self.ps[:, 0:GW])
                        nc.vector.reduce_max(
                            self.st[:, 2 * g:2 * g + 1], lg[:], axis=AX.X, negate=True
                        )
                        nc.scalar.activation(
                            trash[:], lg[:], AF.Exp,
                            bias=self.st[:, 2 * g:2 * g + 1],
                            accum_out=self.st[:, 2 * g + 1:2 * g + 2],
                        )
                        nc.sync.dma_start(
                            logits_d[m * 128:(m + 1) * 128, g * GW:(g + 1) * GW], lg[:]
                        )
                        self.g += 1
                        if self.g == NG:
                            self.g = 0
                            nc.sync.dma_start(stats_d[m], self.st[:])
                            asm_tiles.pop(m, None)
                            self.m += 1

            pe = ProjEmitter()

            # Wo loads gate only the projection; emit them after the small
            # inputs so embt/wi win the DMA queues.
            wo = wpool.tile([128, NJ, VS], BF16)
            for j in range(NJ):
                nc.sync.dma_start(wo[:, j, :], wo_d[j])

            done_m = -1
            for s in range(0, 274):
                # block 0: one step per slot
                if s < T:
                    if s % WLEN[0] == 0:
                        emit_window(0, s // WLEN[0])
                    emit_step(0, s)
                if s >= 16:
                    d = s - 16
                    # block 1: every 2 slots, windowed cross from S0
                    if d % 2 == 0 and d // 2 < CH[1]:
                        tau = d // 2
                        if tau % 8 == 0:
                            emit_window(1, tau // 8)
                            emit_cross_window_b1(tau // 8)
                        emit_step(1, tau)
                    # block 2: every 4 slots
                    if d % 4 == 0 and d // 4 < CH[2]:
                        tau = d // 4
                        if tau % 8 == 0:
                            emit_window(2, tau // 8)
                        emit_step(2, tau)
                    # block 3: every 8 slots
                    if d % 8 == 0 and d // 8 < CH[3]:
                        tau = d // 8
                        if tau % 4 == 0:
                            emit_window(3, tau // 4)
                        emit_step(3, tau)
                if s >= 24 and (s - 24) % 8 == 0 and (s - 24) // 8 < MT:
                    done_m = (s - 24) // 8
                for _ in range(2):
                    if pe.next_m() is None or pe.next_m() > done_m:
                        break
                    pe.emit_sweep()
            while pe.next_m() is not None:
                pe.emit_sweep()

    nc.compile()
    return nc


_NC_CACHE = {}
LAST_RESULT = {}


def _get_nc(use_bias=True):
    key = ("nc", use_bias)
    if key not in _NC_CACHE:
        _NC_CACHE[key] = _build(use_bias)
    return _NC_CACHE[key]


def _maybe_trace_kwargs():
    """When KERNEL_TRACE=1, enable NTFF profiling (dev/test only)."""
    import os
    if os.environ.get("KERNEL_TRACE") != "1":
        return {}
    try:
        import types
        if "antenv.axon_hooks" not in sys.modules:
            mod = types.ModuleType("antenv.axon_hooks")
            state = {"hook": None}
            mod.set_axon_ntff_profile_hook = lambda h: state.__setitem__("hook", h)
            mod.get_axon_ntff_profile_hook = lambda: state["hook"]
            sys.modules["antenv.axon_hooks"] = mod
            from trn_agent_boot.trn_boot import _ntff_profile_via_ctypes
            mod.set_axon_ntff_profile_hook(
                _ntff_profile_via_ctypes("/opt/axon/libaxon_pjrt.so")
            )
        return {"trace": True}
    except Exception:
        return {}


def kernel(x, x_sl, Wi, Wh, Wo, bi, bh, initial_state, embedding):
    x = np.asarray(x)
    x_sl = np.asarray(x_sl)
    Wi = np.asarray(Wi, np.float32)
    Wh = np.asarray(Wh, np.float32)
    Wo = np.asarray(Wo, np.float32)
    bi = np.asarray(bi, np.float32)
    bh = np.asarray(bh, np.float32)
    initial_state = np.asarray(initial_state, np.float32)
    embedding = np.asarray(embedding, np.float32)

    xin = x[:, :-1].astype(np.int64)
    y = x[:, 1:].astype(np.int64)
    sl = (x_sl - 1).astype(np.int64)

    bf16 = ml_dtypes.bfloat16
    emb = embedding[xin]                                   # [B,T,E] f32
    embt = np.ascontiguousarray(
        emb.transpose(2, 1, 0).reshape(EMB, ROWS)          # col = t*16+b
    ).reshape(NE, 128, ROWS).astype(bf16)
    wi_h = Wi.reshape(NE, 128, NH).astype(bf16)
    bsumb_h = (bi + bh).reshape(1, NJ, 128).astype(bf16)
    whb_h = np.stack(
        [Wh[ji * 128:(ji + 1) * 128, jo * 128:(jo + 1) * 128] for jo, ji in WH_PAIRS]
    ).astype(bf16)
    hinit_h = np.ascontiguousarray(
        np.broadcast_to(initial_state.reshape(NJ, 128).T[:, :, None], (128, NJ, B))
    ).astype(bf16)
    wo_r = Wo.reshape(NJ, 128, VOCAB).astype(bf16)

    base = {
        "embt": embt, "wi": wi_h, "bsumb": bsumb_h, "whb": whb_h, "hinit": hinit_h,
        "ones": np.ones((1, 256), dtype=bf16),
    }
    in_maps = [
        {**base, "wo": np.ascontiguousarray(wo_r[:, :, c * VS:(c + 1) * VS])}
        for c in range(NCORES)
    ]

    nc = _get_nc(use_bias=bool(np.any(bsumb_h)))
    res = bass_utils.run_bass_kernel_spmd(
        nc, in_maps, core_ids=list(range(NCORES)), **_maybe_trace_kwargs()
    )
    LAST_RESULT["res"] = res

    logits_rows = np.concatenate(
        [res.results[c]["logits"] for c in range(NCORES)], axis=1
    )                                                      # [ROWS, VOCAB]
    stats = np.stack([res.results[c]["stats"] for c in range(NCORES)])
    # stats[c, m, p, 2g] = -max, [.., 2g+1] = sumexp ; row = m*128+p
    negmax = stats[:, :, :, 0::2].reshape(NCORES, ROWS, NG)
    sume = stats[:, :, :, 1::2].reshape(NCORES, ROWS, NG)
    M_cg = -negmax.astype(np.float64)
    S_cg = sume.astype(np.float64)
    M = M_cg.max(axis=(0, 2))                              # [ROWS]
    Ssum = (np.exp(M_cg - M[None, :, None]) * S_cg).sum(axis=(0, 2))
    lse = M + np.log(Ssum)                                 # [ROWS]

    y_row = y.T.reshape(ROWS)                              # row = t*16+b
    tgt = logits_rows[np.arange(ROWS), y_row].astype(np.float64)
    ll = tgt - lse
    t_idx = np.arange(ROWS) // B
    b_idx = np.arange(ROWS) % B
    mask = t_idx < sl[b_idx]
    loss = np.float32(-(ll * mask).sum() / sl.sum())

    logits = np.ascontiguousarray(
        logits_rows.reshape(T, B, VOCAB).transpose(1, 0, 2)
    )
    return loss, logits
